# revision 47
# baseline (speedup 1.0000x reference)
"""Trainium2 Bass kernel for MemoryAsContextTransformer segmented attention.

Reference computation (per full input):
  h   = rmsnorm(x, gamma)                      [B=2, S=4096, D=1024]
  qkv = h @ w_qkv                              heads=16, dh=64, seg=512, pm=16
  per (batch, segment, head): block-causal attention with 16 persistent
  memory tokens prepended to k/v, softmax, out = attn @ v
  out @ w_out                                  [2, 4096, 1024]

Sharding: data-parallel over the 16 (batch, segment) units; 2 contiguous
segments (1024 tokens) per core, full weights broadcast to all 8 cores.

v2 design (per core), all matmul operands fp16 (PSUM accumulates f32;
fp8/bf16 fail the 2e-2 error budget and DoubleRow fp8 is only ~1.44x on HW):
  A: x (fp16, host-pretiled [p, tt, d]); sumsq via ACT Square+accumulator
     (keeps DVE free at startup); rs = 1/sqrt(mean+eps); hT = x^T scaled in
     one shot by a PE matmul with diag(rs) as the moving operand.
  B: q/k projections, two ot outputs per 2-bank PSUM tile; gamma and
     dh^-0.5 folded into the host-pretiled weights.
  C: v projection, row layout v[j, tt, head, 128] with ones in cols 64:127
     (memset once) so PV replicates the softmax denominator 64x.
  D: per (seg): persistent-memory scores for 2-3 head-pairs per PSUM bank
     via 32-row band matmuls (zero-padded stacked stationary), one exp per
     bank. Per head-PAIR: causally-restricted QK^T blocks into one 3-bank
     tile per head, the pair's matmuls emitted adjacently so the disjoint
     64-row PE groups run concurrently on HW; ONE exp per head; causal mask
     as a multiplicative 0/1 upper-triangle on Pool (SBUF only - GPSIMD
     cannot touch PSUM); PV with the ones columns yields [attnT(64);
     den x64] in one bank; normalize = DVE reciprocal (PSUM->SBUF) + one
     PSUM x SBUF multiply (HW allows only one PSUM operand per op).
  E: out projection, interleaved into D(seg1)'s pipeline for seg0 and
     appended for seg1, quarter-tile copies + DMAs to shorten the tail.
  Schedule: A; B/C(tch0); D(seg0) interleaved with B/C(tch1); D(seg1)
  interleaved with E(seg0); E(seg1). Everything lives in one pool scope;
  PSUM = 2x 3-bank sim tiles + 2x 1-bank pv/pm/E ring. All weights are
  host-pretiled so every DMA is contiguous per partition, issued on SP in
  consumer order (a DMA occupies its queue for the whole transfer).
"""

import sys

sys.path.insert(0, "/opt/trn_rl_repo")

from contextlib import ExitStack

import numpy as np

import concourse.mybir as mybir
import concourse.tile as tile
from concourse import bacc

F32 = mybir.dt.float32
F16 = mybir.dt.float16
AF = mybir.ActivationFunctionType
OP = mybir.AluOpType

B, S, D = 2, 4096, 1024
HEADS, DH, SEG, PM = 16, 64, 512, 16
INNER = HEADS * DH          # 1024
NCORES = 8
TOK = (B * S) // NCORES     # 1024 tokens per core
NSEG = TOK // SEG           # 2 segments per core
TT = TOK // 128             # 8 token tiles
NI2 = INNER // 128          # 8 inner tiles
KT8 = D // 128              # 8 matmul k-tiles
EPS = 1e-6
SW = 1.0                    # weight prescale (1.0: fp16 needs none)


def build_bass(repeat=1):
    nc = bacc.Bacc("TRN2", target_bir_lowering=False, debug=False)

    x_d = nc.dram_tensor("x16", [128, TT, D], F16, kind="ExternalInput")
    w8_d = nc.dram_tensor("w16", [128, 2 * NI2, KT8, 128], F16, kind="ExternalInput")
    wv8_d = nc.dram_tensor("wv16", [128, KT8, INNER], F16, kind="ExternalInput")
    wo8_d = nc.dram_tensor("wo16", [128, KT8, D], F16, kind="ExternalInput")
    pmst_d = nc.dram_tensor("pmst", [128, NI2, 32], F16, kind="ExternalInput")
    pmvo_d = nc.dram_tensor("pmvo", [128, HEADS, 128], F16, kind="ExternalInput")
    tri4_d = nc.dram_tensor("tri4", [128, 512], F16, kind="ExternalInput")
    ident_d = nc.dram_tensor("ident", [128, 128], F16, kind="ExternalInput")
    o_d = nc.dram_tensor("o", [TOK, D], F32, kind="ExternalOutput")

    with tile.TileContext(nc) as tc:
     for _rep in range(repeat):
      with ExitStack() as octx:
        consts = octx.enter_context(tc.tile_pool(name="consts", bufs=1))
        big = octx.enter_context(tc.tile_pool(name="big", bufs=1))

        # ---- prefetch everything; spread issues over SP/Act/Pool DGE paths so
        # transfers run in parallel and the first consumers unblock early
        x_sb = big.tile([128, TT, D], F16)
        ident_sb = consts.tile([128, 128], F16)
        w8_sb = big.tile([128, 2 * NI2, KT8, 128], F16)
        wv8_sb = big.tile([128, KT8, INNER], F16)
        wo8_sb = big.tile([128, KT8, D], F16)
        pmst_sb = consts.tile([128, NI2, 32], F16)
        pmvo_sb = consts.tile([128, HEADS, 128], F16)
        tri4_sb = consts.tile([128, 512], F16)
        eps_sb = consts.tile([128, 1], F32)
        nc.vector.memset(eps_sb[:], EPS)
        # a DMA occupies its issuing engine's queue for the whole transfer, so
        # everything goes on SP, interleaved so early consumers unblock first
        for tt in range(4):
            nc.sync.dma_start(x_sb[:, tt, :], x_d[:, tt, :])
            if tt == 0:
                nc.sync.dma_start(ident_sb[:], ident_d[:])
        for tt in range(4, TT):
            c = 2 * (tt - 4)
            nc.sync.dma_start(w8_sb[:, c : c + 2], w8_d[:, c : c + 2])
            nc.sync.dma_start(x_sb[:, tt, :], x_d[:, tt, :])
        for c in range(8, 16, 2):
            nc.sync.dma_start(w8_sb[:, c : c + 2], w8_d[:, c : c + 2])
        nc.sync.dma_start(wv8_sb[:], wv8_d[:])
        nc.sync.dma_start(pmst_sb[:], pmst_d[:])
        nc.sync.dma_start(pmvo_sb[:], pmvo_d[:])
        nc.sync.dma_start(tri4_sb[:], tri4_d[:])
        nc.sync.dma_start(wo8_sb[:], wo8_d[:])

        hT = big.tile([128, NI2, TOK], F16)          # h^T, d on partitions
        qkT = big.tile([128, 2 * NI2, TOK], F16)   # ot 0..7 q (pre-scaled), 8..15 k
        v_sb = big.tile([128, TT, HEADS, 128], F16)
        aoT = big.tile([128, NI2, TOK], F16)         # unnormalized -> normalized attnT
        o_pool = octx.enter_context(tc.tile_pool(name="o", bufs=2))

        def emit_E(tt, pool, tag, engs):
            o_sb = o_pool.tile([128, D], F32, tag="osb")
            for ech in range(2):
                ps = pool.tile([128, 512], F32, tag=tag)
                for kt in range(KT8):
                    nc.tensor.matmul(
                        ps[:],
                        aoT[:, kt, tt * 128 : (tt + 1) * 128],
                        wo8_sb[:, kt, ech * 512 : (ech + 1) * 512],
                        start=(kt == 0), stop=(kt == KT8 - 1),
                    )
                for q in range(2):
                    lo, hi = ech * 512 + q * 256, ech * 512 + (q + 1) * 256
                    if (ech + q) % 2 == 0:
                        nc.vector.tensor_copy(o_sb[:, lo:hi], ps[:, q * 256 : (q + 1) * 256])
                    else:
                        nc.scalar.activation(o_sb[:, lo:hi], ps[:, q * 256 : (q + 1) * 256], AF.Copy)
                    nc.sync.dma_start(
                        o_d[tt * 128 : (tt + 1) * 128, lo:hi], o_sb[:, lo:hi])

        with ExitStack() as actx:
            stat = actx.enter_context(tc.tile_pool(name="stat", bufs=6))
            rec_pool = actx.enter_context(tc.tile_pool(name="rec", bufs=2))
            pp_pool = actx.enter_context(tc.tile_pool(name="pp", bufs=3))
            p3_pool = actx.enter_context(tc.tile_pool(name="p3", bufs=3))
            # 3-bank tiles: per-head sim blocks; A transposes and B/C use them too
            ps_sim = actx.enter_context(tc.tile_pool(name="ps_sim", bufs=2, space="PSUM"))
            # 1-bank ring: pv tiles, pm scores, E tiles
            ps_pv = actx.enter_context(tc.tile_pool(name="ps_pv", bufs=2, space="PSUM"))

            # ---- Phase A: rmsnorm + transpose via diag(rs) matmul. sumsq on
            # ACT (Square + free-dim accumulator) keeps DVE free at startup.
            def emit_A(tt):
                sq = stat.tile([128, 1], F32, tag="sq")
                # Square's elementwise output is never read; dump it into the
                # aoT region (overwritten by the normalize much later)
                with nc.allow_low_precision(reason="x^2 scratch is unused"):
                    nc.scalar.activation(
                        aoT[:, :, tt * 128 : (tt + 1) * 128],
                        x_sb[:, tt, :].rearrange("p (a c) -> p a c", c=128),
                        AF.Square, accum_out=sq[:])
                s_t = stat.tile([128, 1], F32, tag="s")
                nc.scalar.activation(s_t[:], sq[:], AF.Sqrt, bias=eps_sb[:], scale=1.0 / D)
                rs_t = stat.tile([128, 1], F32, tag="rs")
                nc.vector.reciprocal(rs_t[:], s_t[:])
                diag = stat.tile([128, 128], F16, tag="diag")
                with nc.allow_low_precision(reason="fp16 diag for transpose"):
                    nc.vector.tensor_scalar_mul(diag[:], ident_sb[:], rs_t[:])
                ptr = ps_sim.tile([128, 1536], F32, tag="sim")
                for half in range(2):
                    for i in range(4):
                        db = half * 4 + i
                        nc.tensor.matmul(
                            ptr[:, half * 512 + i * 128 : half * 512 + (i + 1) * 128],
                            x_sb[:, tt, db * 128 : (db + 1) * 128],
                            diag[:],
                            start=(i == 0), stop=(i == 3),
                            skip_group_check=True,
                        )
                with nc.allow_low_precision(reason="h stored fp16"):
                    nc.scalar.activation(
                        hT[:, 0:4, tt * 128 : (tt + 1) * 128],
                        ptr[:, 0:512].rearrange("p (f c) -> p f c", c=128),
                        AF.Copy,
                    )
                    nc.vector.tensor_copy(
                        hT[:, 4:8, tt * 128 : (tt + 1) * 128],
                        ptr[:, 512:1024].rearrange("p (f c) -> p f c", c=128),
                    )

            # A(tt0-3) unblocks B(tch0); emitting B before A(tt4-7) keeps the
            # in-order PE stream fed while the late tiles' rms chains resolve
            for tt in range(4):
                emit_A(tt)

            def emit_B(tch, otp, use_act):
                # tch0 runs before attention and shares the big sim ring;
                # tch1 interleaves with D(seg0), so it uses the 1-bank pv
                # ring instead of contending with the head-pair sim tiles
                if use_act:
                    big_ps = ps_sim.tile([128, 1536], F32, tag="sim")
                    halves = [big_ps[:, 0:512], big_ps[:, 512:1024]]
                else:
                    halves = []
                    for half in range(2):
                        ps_h = ps_pv.tile([128, 512], F32, tag="pv")
                        halves.append(ps_h[:])
                for half in range(2):
                    ot = 2 * otp + half
                    for kt in range(KT8):
                        nc.tensor.matmul(
                            halves[half],
                            w8_sb[:, ot, kt],
                            hT[:, kt, tch * 512 : (tch + 1) * 512],
                            start=(kt == 0), stop=(kt == KT8 - 1),
                            skip_group_check=True,
                        )
                for half in range(2):
                    ot = 2 * otp + half
                    out_ap = qkT[:, ot, tch * 512 : (tch + 1) * 512]
                    eng = [nc.scalar, nc.vector][ot % 2]
                    with nc.allow_low_precision(reason="qk stored fp16"):
                        if eng is nc.scalar:
                            nc.scalar.activation(out_ap, halves[half], AF.Copy, scale=1.0 / SW)
                        else:
                            eng.tensor_scalar_mul(out_ap, halves[half], 1.0 / SW)

            def emit_C(tt):
                if tt < 4:
                    big_ps = ps_sim.tile([128, 1536], F32, tag="sim")
                    chalves = [big_ps[:, 0:512], big_ps[:, 512:1024]]
                else:
                    chalves = []
                    for och in range(2):
                        ps_h = ps_pv.tile([128, 512], F32, tag="pv")
                        chalves.append(ps_h[:])
                for och in range(2):
                    for kt in range(KT8):
                        nc.tensor.matmul(
                            chalves[och],
                            hT[:, kt, tt * 128 : (tt + 1) * 128],
                            wv8_sb[:, kt, och * 512 : (och + 1) * 512],
                            start=(kt == 0), stop=(kt == KT8 - 1),
                            skip_group_check=True,
                        )
                for och in range(2):
                    eng = [nc.vector, nc.scalar][(2 * tt + och) % 2]
                    with nc.allow_low_precision(reason="v stored fp16"):
                        if eng is nc.scalar:
                            nc.scalar.activation(
                                v_sb[:, tt, och * 8 : (och + 1) * 8, 0:DH],
                                chalves[och].rearrange("p (h o) -> p h o", o=DH),
                                AF.Copy,
                            )
                        else:
                            eng.tensor_scalar_mul(
                                v_sb[:, tt, och * 8 : (och + 1) * 8, 0:DH],
                                chalves[och].rearrange("p (h o) -> p h o", o=DH),
                                1.0 / SW,
                            )

            # pm scores: 2-3 ot-groups (4-6 heads) per PSUM bank via 32-row
            # band matmuls (matmul partition bases must be in {0,32,64})
            OT_GROUPS = [[0, 1, 2], [3, 4, 5], [6, 7]]

            def emit_pm(seg):
                pps = []
                for group in OT_GROUPS:
                    pmps = ps_pv.tile([128, 512], F32, tag="pv")
                    for g, ot in enumerate(group):
                        nc.tensor.matmul(
                            pmps[32 * g : 32 * g + 32, :],
                            pmst_sb[:, ot, :],
                            qkT[:, ot, seg * 512 : (seg + 1) * 512],
                            start=True, stop=True,
                            skip_group_check=True,
                        )
                    pp = pp_pool.tile([128, 512], F16, tag="pp")
                    nrow = 32 * len(group)
                    with nc.allow_low_precision(reason="softmax weights fp16"):
                        nc.scalar.activation(pp[0:nrow, :], pmps[0:nrow, :], AF.Exp)
                    pps.append(pp)
                return pps

            def emit_head_pair(seg, ot, pps):
                # both heads of ot: even head in PE rows 0:64, odd in 64:128.
                # Their QK matmuls are emitted adjacently per key block so the
                # PE runs the disjoint row-groups concurrently (tile_position
                # packing; ~2x QK throughput on HW).
                b = min(ot // 3, 2)
                g = ot - 3 * b
                T0 = seg * 4
                sims, p3s = [], []
                for sub in range(2):
                    sim_m = ps_sim.tile([128, 1536], F32, tag="sim")
                    sims.append(sim_m)
                qs = [qkT[pb : pb + 64, ot, seg * 512 : (seg + 1) * 512]
                      for pb in (0, 64)]
                ks = [qkT[pb : pb + 64, NI2 + ot, seg * 512 : (seg + 1) * 512]
                      for pb in (0, 64)]
                # key blocks tj0 | tj1+tj3 | tj2 packed into one 3-bank tile
                for dst, ksl, qsl, start in (
                    ((0, 512), (0, 128), (0, 512), True),
                    ((512, 896), (128, 256), (128, 512), True),
                    ((896, 1024), (384, 512), (384, 512), False),
                    ((1024, 1280), (256, 384), (256, 512), True),
                ):
                    for sub in range(2):
                        nc.tensor.matmul(
                            sims[sub][:, dst[0] : dst[1]],
                            ks[sub][:, ksl[0] : ksl[1]],
                            qs[sub][:, qsl[0] : qsl[1]],
                            start=start, stop=False, skip_group_check=True)
                for sub in range(2):
                    p3 = p3_pool.tile([128, 1536], F16, tag="p3")
                    p3s.append(p3)
                    with nc.allow_low_precision(reason="softmax weights fp16"):
                        nc.scalar.activation(p3[:, 0:1280], sims[sub][:, 0:1280], AF.Exp)
                        # causal mask: zero upper-triangle of the 4 diagonal
                        # 128-blocks (p3 cols 0, 512, 896, 1024)
                        p3v = p3[:].rearrange("p (a c) -> p a c", c=512)
                        nc.gpsimd.tensor_mul(
                            p3v[:, :, 0:128], p3v[:, :, 0:128], tri4_sb[:, 0:384])
                        nc.gpsimd.tensor_mul(
                            p3[:, 896:1024], p3[:, 896:1024], tri4_sb[:, 0:128])
                for sub in range(2):
                    h, pb, p3 = 2 * ot + sub, 64 * sub, p3s[sub]
                    pv = ps_pv.tile([128, 512], F32, tag="pv")
                    nc.tensor.matmul(pv[:], pmvo_sb[32 * g : 32 * g + 32, h, :],
                                     pps[b][32 * g : 32 * g + 32, :],
                                     start=True, stop=False, skip_group_check=True)
                    nc.tensor.matmul(pv[:, 0:512], v_sb[:, T0 + 0, h, :], p3[:, 0:512],
                                     start=False, stop=False, skip_group_check=True)
                    nc.tensor.matmul(pv[:, 128:512], v_sb[:, T0 + 1, h, :], p3[:, 512:896],
                                     start=False, stop=False, skip_group_check=True)
                    nc.tensor.matmul(pv[:, 384:512], v_sb[:, T0 + 3, h, :], p3[:, 896:1024],
                                     start=False, stop=False, skip_group_check=True)
                    nc.tensor.matmul(pv[:, 256:512], v_sb[:, T0 + 2, h, :], p3[:, 1024:1280],
                                     start=False, stop=True, skip_group_check=True)
                    # only one matmul operand may live in PSUM per DVE op:
                    # reciprocal of the replicated denominator rows -> SBUF,
                    # then one PSUM x SBUF multiply writes normalized attnT
                    rec = rec_pool.tile([DH, 512], F16, tag="rec")
                    with nc.allow_low_precision(reason="attn out fp16"):
                        nc.vector.reciprocal(rec[:], pv[DH:128, :])
                        nc.vector.tensor_mul(
                            aoT[pb : pb + 64, ot, seg * 512 : (seg + 1) * 512],
                            pv[0:DH, :], rec[:],
                        )

            # ---- schedule: A(0-3); B(tch0); A(4-7); C(tch0); D(seg0)
            # interleaved with B/C(tch1); D(seg1) interleaved with E(seg0)
            for otp in range(NI2):
                emit_B(0, otp, use_act=True)
            for tt in range(4, TT):
                emit_A(tt)
            # ones for the PV denominator rows; Pool queue, after the hT copies
            nc.gpsimd.memset(v_sb[:, :, :, DH:128].bitcast(F16), 1.0)
            pps = emit_pm(0)
            for tt in range(4):
                emit_C(tt)
            for ot in range(NI2):
                emit_head_pair(0, ot, pps)
                emit_B(1, ot, use_act=False)
                if ot % 2 == 1:
                    emit_C(4 + ot // 2)
            pps = emit_pm(1)
            for ot in range(NI2):
                if ot % 2 == 1 and ot < 7:
                    emit_E(ot // 2, ps_pv, "pv", (nc.vector, nc.gpsimd))
                emit_head_pair(1, ot, pps)
            emit_E(3, ps_pv, "pv", (nc.vector, nc.gpsimd))

            # out projection for seg1's token tiles (same pv ring: no new
            # PSUM pool, no cross-scope bank-reuse serialization)
            for tt in range(4, TT):
                emit_E(tt, ps_pv, "pv", (nc.vector, nc.gpsimd))

    nc.compile()
    return nc


_NC_CACHE = None


def _get_nc():
    global _NC_CACHE
    if _NC_CACHE is None:
        _NC_CACHE = build_bass()
    return _NC_CACHE


class _Runner:
    """Compile the Bass program once into a sharded jitted callable over the
    8 NeuronCores; reuse it for every kernel() invocation."""

    def __init__(self, nc):
        import jax
        from jax.sharding import Mesh, PartitionSpec
        from jax.experimental.shard_map import shard_map
        from concourse import bass2jax

        bass2jax.install_neuronx_cc_hook()
        self.nc = nc
        pname = nc.partition_id_tensor.name if nc.partition_id_tensor else None
        in_names, out_names, out_avals, self.zero_shapes = [], [], [], []
        for alloc in nc.m.functions[0].allocations:
            if not isinstance(alloc, mybir.MemoryLocationSet):
                continue
            name = alloc.memorylocations[0].name
            if alloc.kind == "ExternalInput":
                if name != pname:
                    in_names.append(name)
            elif alloc.kind == "ExternalOutput":
                out_names.append(name)
                shape = tuple(alloc.tensor_shape)
                dtype = mybir.dt.np(alloc.dtype)
                out_avals.append(jax.core.ShapedArray(shape, dtype))
                self.zero_shapes.append((shape, dtype))
        self.in_names, self.out_names = in_names, out_names
        all_in = in_names + out_names + ([pname] if pname else [])

        def _body(*args):
            operands = list(args)
            if pname is not None:
                operands.append(bass2jax.partition_id_tensor())
            return tuple(
                bass2jax._bass_exec_p.bind(
                    *operands,
                    out_avals=tuple(out_avals),
                    in_names=tuple(all_in),
                    out_names=tuple(out_names),
                    lowering_input_output_aliases=(),
                    sim_require_finite=True,
                    sim_require_nnan=True,
                    nc=nc,
                )
            )

        devices = jax.devices()[:NCORES]
        self.mesh = Mesh(np.asarray(devices), ("core",))
        self.sharding = jax.sharding.NamedSharding(self.mesh, PartitionSpec("core"))
        n_params = len(in_names)
        donate = tuple(range(n_params, n_params + len(out_names)))
        self.sharded = jax.jit(
            shard_map(
                _body,
                mesh=self.mesh,
                in_specs=(PartitionSpec("core"),) * (n_params + len(out_names)),
                out_specs=(PartitionSpec("core"),) * len(out_names),
                check_rep=False,
            ),
            donate_argnums=donate,
            keep_unused=True,
        )
        self._jax = jax

    def device_inputs(self, in_maps):
        concat = [
            np.concatenate([np.asarray(m[nm]) for m in in_maps], axis=0)
            for nm in self.in_names
        ]
        return [self._jax.device_put(a, self.sharding) for a in concat]

    def zeros(self):
        return [
            self._jax.device_put(
                np.zeros((NCORES * s[0], *s[1:]), d), self.sharding
            )
            for s, d in self.zero_shapes
        ]

    def __call__(self, dev_in):
        outs = self.sharded(*dev_in, *self.zeros())
        for o in outs:
            o.block_until_ready()
        return outs


_RUNNER = None


def _get_runner():
    global _RUNNER
    if _RUNNER is None:
        _RUNNER = _Runner(_get_nc())
    return _RUNNER


def make_in_maps(x, gamma, w_qkv, w_out, pm_k, pm_v):
    F16H = np.float16
    x = np.asarray(x, dtype=np.float32).reshape(B * S, D)
    gamma = np.asarray(gamma, dtype=np.float32)
    w_qkv = np.asarray(w_qkv, dtype=np.float32)
    w_out = np.asarray(w_out, dtype=np.float32)
    pm_k = np.asarray(pm_k, dtype=np.float32)
    pm_v = np.asarray(pm_v, dtype=np.float32)

    w = w_qkv * gamma[:, None]                       # fold gamma into projections
    scale = DH ** -0.5
    # q (scaled), k: [D, 2*INNER] -> [p, ot, kt, two, oc]
    w_qk = np.concatenate([w[:, :INNER] * scale, w[:, INNER : 2 * INNER]], axis=1)
    w8 = np.ascontiguousarray(
        w_qk.reshape(KT8, 128, 2 * NI2, 128).transpose(1, 2, 0, 3)
    ).astype(F16H)
    # v: [D, INNER] -> [p, kt, o]
    wv8 = np.ascontiguousarray(
        w[:, 2 * INNER :].reshape(KT8, 128, INNER).transpose(1, 0, 2)
    ).astype(F16H)
    # out: [INNER, D] -> [p, kt, e]
    wo8 = np.ascontiguousarray(
        w_out.reshape(KT8, 128, D).transpose(1, 0, 2)
    ).astype(F16H)

    # pm_k stacked stationary: [128, NI2, 32]; head 2ot at partitions 0:64
    # cols 0:16, head 2ot+1 at partitions 64:128 cols 16:32
    pmst = np.zeros((128, NI2, 32), dtype=np.float32)
    for ot in range(NI2):
        pmst[0:64, ot, 0:16] = pm_k[2 * ot].T
        pmst[64:128, ot, 16:32] = pm_k[2 * ot + 1].T
    pmst = pmst.astype(F16H)

    # pm_v + ones: [128, HEADS, 128]; head h in band 32g of its ot-group bank
    # (groups [0,1,2], [3,4,5], [6,7]), sub-rows 16*(h%2)
    pmvo = np.zeros((128, HEADS, 128), dtype=np.float32)
    for h in range(HEADS):
        ot = h // 2
        b = min(ot // 3, 2)
        g = ot - 3 * b
        r0 = 32 * g + 16 * (h % 2)
        pmvo[r0 : r0 + 16, h, 0:DH] = pm_v[h]
        pmvo[r0 : r0 + 16, h, DH:128] = 1.0
    pmvo = pmvo.astype(F16H)

    r = np.arange(128)
    # tri4[j, i] = 1 if j <= i else 0, tiled 4x: multiplicative causal mask for
    # the diagonal 128-blocks of p (keys j after query i get zeroed post-exp)
    tri = (r[:, None] <= r[None, :]).astype(np.float32)
    tri4 = np.tile(tri, (1, 4)).astype(F16H)
    ident = np.eye(128, dtype=np.float32).astype(F16H)

    x8 = np.ascontiguousarray(
        x.reshape(NCORES, TT, 128, D).transpose(0, 2, 1, 3)
    ).astype(F16H)

    shared = {
        "w16": w8,
        "wv16": wv8,
        "wo16": wo8,
        "pmst": pmst,
        "pmvo": pmvo,
        "tri4": tri4,
        "ident": ident,
    }
    return [
        {"x16": np.ascontiguousarray(x8[c]), **shared}
        for c in range(NCORES)
    ]


def kernel(x, gamma, w_qkv, w_out, pm_k, pm_v):
    runner = _get_runner()
    in_maps = make_in_maps(x, gamma, w_qkv, w_out, pm_k, pm_v)
    outs = runner(runner.device_inputs(in_maps))
    out = np.asarray(outs[0])          # [NCORES*TOK, D] global row-sharded
    return out.reshape(B, S, D)


if __name__ == "__main__":
    rng = np.random.default_rng(0)
    ins = {
        "x": rng.standard_normal((B, S, D), dtype=np.float32),
        "gamma": np.ones(D, dtype=np.float32),
        "w_qkv": (rng.standard_normal((D, 3 * INNER), dtype=np.float32) * D**-0.5),
        "w_out": (rng.standard_normal((INNER, D), dtype=np.float32) * INNER**-0.5),
        "pm_k": (rng.standard_normal((HEADS, PM, DH), dtype=np.float32) * 0.02),
        "pm_v": (rng.standard_normal((HEADS, PM, DH), dtype=np.float32) * 0.02),
    }
    out = kernel(**ins)
    print("out", out.shape, out.dtype, np.abs(out).mean())
